# revision 1
# baseline (speedup 1.0000x reference)
"""Poker fused embedding kernel for 8x TRN2 NeuronCores (Bass/Tile).

Strategy:
  - Host: shard batch across 8 cores (16 rows each -> 16384 tokens/core).
    Sort each core's tokens into segments [CLS | plain | card | action | ctx],
    excluding padding tokens (their output rows are zero).  Pad each segment
    to a multiple of 128 tokens with dummy tokens; tile counts are maxed
    across cores so all cores run one SPMD program.
  - Device: per 128-token tile, build a one-hot matrix via a tiny broadcast
    matmul + is_equal compare, then gather all embedding-table contributions
    with bf16 matmuls against a per-category combined table (hi/lo split for
    fp32-grade accuracy).  Action/context tiles additionally run the 16->256
    MLP as matmuls (bias via ones-row), LayerNorm via bn_stats and a fused
    ACT Relu with per-partition scale/bias.
  - Host: scatter the compacted per-core outputs back to [B,S,D].
"""
import numpy as np
import ml_dtypes

import concourse.bacc as bacc
import concourse.tile as tile
from concourse import mybir
from concourse.bass_utils import run_bass_kernel_spmd
from concourse.tile_rust import add_dep_helper

F32 = mybir.dt.float32
BF16 = mybir.dt.bfloat16
AF = mybir.ActivationFunctionType
ALU = mybir.AluOpType
NPBF = ml_dtypes.bfloat16

# problem constants
NBB = 16
D = 256
CARD_OFF = 8
ACTION_OFF = 60
CONTEXT_ID = 1
PAD = 76
NCTX = 16
B, S = 128, 1024
NCORES = 8
TPC = (B // NCORES) * S    # tokens per core
TILE = 128
GRP = 4                    # tiles per matmul/DMA group
MISS = 999.0               # never matches any iota entry (1000.0 in bf16)

K_CARD = 77 + 4 + 13 + 4           # 98
K_ACT = 77 + 16 + 4 + 2            # 99
K_PLAIN = 77 + 4                   # 81
K_CLS = 77 + 16 + 4 + 13 + 4 + 2   # 116
KMAX = 99


def _hi_lo(x):
    hi = x.astype(NPBF)
    lo = (x - hi.astype(np.float32)).astype(NPBF)
    return hi, lo


def _build_host_data(token_ids, token_streets, card_ranks, card_suits,
                     action_actors, action_legal_masks, context_features):
    ids = token_ids.reshape(-1)
    streets = token_streets.reshape(-1)
    ranks = card_ranks.reshape(-1)
    suits = card_suits.reshape(-1)
    actors = action_actors.reshape(-1)
    masks = action_legal_masks.reshape(-1, NBB)
    ctxf = context_features.reshape(-1, NCTX)

    cores = []
    for c in range(NCORES):
        lo = c * TPC
        idx = np.arange(lo, lo + TPC)
        cid = ids[idx]
        is_cls = (idx % S) == 0
        is_pad = cid < 0
        is_ctx = cid == CONTEXT_ID
        is_card = (cid >= CARD_OFF) & (cid < ACTION_OFF)
        is_act = (cid >= ACTION_OFF) & (cid < PAD)
        rest = ~is_cls & ~is_pad
        cores.append(dict(
            cls=idx[is_cls],
            plain=idx[rest & ~is_ctx & ~is_card & ~is_act],
            card=idx[rest & is_card],
            act=idx[rest & is_act],
            ctx=idx[rest & is_ctx]))

    ntiles = {k: max((len(cc[k]) + TILE - 1) // TILE for cc in cores)
              for k in ("plain", "card", "act", "ctx")}

    def pad_seg(seg, n_tiles):
        out = np.full(n_tiles * TILE, -1, dtype=np.int64)
        out[: len(seg)] = seg
        return out

    per_core = []
    for c in range(NCORES):
        cc = cores[c]
        slots = np.concatenate([
            pad_seg(cc["cls"], 1),
            pad_seg(cc["plain"], ntiles["plain"]),
            pad_seg(cc["card"], ntiles["card"]),
            pad_seg(cc["act"], ntiles["act"]),
            pad_seg(cc["ctx"], ntiles["ctx"]),
        ])
        valid = slots >= 0
        sl = np.where(valid, slots, 0)

        ids_p = np.where(valid, ids[sl], PAD).astype(np.float32)
        street_p = np.where(valid, streets[sl], MISS).astype(np.float32)
        rank_p = np.where(valid, ranks[sl], MISS).astype(np.float32)
        suit_p = np.where(valid, suits[sl], MISS).astype(np.float32)
        actor_p = np.where(valid, actors[sl], MISS).astype(np.float32)

        # CLS tile (slots 0..127): eff values, invalid sections -> MISS
        cls_sl = slots[:TILE]
        cv = cls_sl >= 0
        csl = np.where(cv, cls_sl, 0)
        cid = ids[csl]
        c_pad = (cid < 0) | ~cv
        c_card = (cid >= CARD_OFF) & (cid < ACTION_OFF) & ~c_pad
        c_act = (cid >= ACTION_OFF) & (cid < PAD) & ~c_pad
        ids_p[:TILE] = np.where(c_pad, PAD, cid).astype(np.float32)
        street_p[:TILE] = np.where(cv, streets[csl], MISS).astype(np.float32)
        rank_p[:TILE] = np.where(c_card, ranks[csl], MISS).astype(np.float32)
        suit_p[:TILE] = np.where(c_card, suits[csl], MISS).astype(np.float32)
        actor_p[:TILE] = np.where(c_act, actors[csl], MISS).astype(np.float32)

        bf = lambda a: np.ascontiguousarray(a.astype(NPBF))
        a_act = bf(np.stack([actor_p, ids_p, street_p]))
        a_card = bf(np.stack([ids_p, street_p, rank_p, suit_p]))
        a_plain = bf(np.stack([ids_p, street_p]))
        in_cls = bf(np.stack([actor_p, ids_p, street_p, rank_p,
                              suit_p])[:, :TILE])

        # action segment legal masks (transposed) + ones row (exact in bf16)
        act_lo = TILE * (1 + ntiles["plain"] + ntiles["card"])
        na = ntiles["act"] * TILE
        aslots = slots[act_lo: act_lo + na]
        av = aslots >= 0
        asl = np.where(av, aslots, 0)
        m = np.where(av[:, None], masks[asl], 0.0)
        masksT = bf(np.concatenate([m.T, np.ones((1, na))]))

        # ctx segment features (transposed, hi/lo) + ones row
        ctx_lo = act_lo + na
        nx = ntiles["ctx"] * TILE
        xslots = slots[ctx_lo: ctx_lo + nx]
        xv = xslots >= 0
        xsl = np.where(xv, xslots, 0)
        xf = np.where(xv[:, None], ctxf[xsl], 0.0)
        xT = np.concatenate([xf.T, np.ones((1, nx))]).astype(np.float32)
        ctxT_hi, ctxT_lo = _hi_lo(xT)

        # CLS-tile aux
        m_cls = np.where(cv[:, None], masks[csl], 0.0)
        masksT_cls = bf(np.concatenate([m_cls.T, np.ones((1, TILE))]))
        x_cls = np.where(cv[:, None], ctxf[csl], 0.0)
        xclsT = np.concatenate([x_cls.T, np.ones((1, TILE))]).astype(np.float32)
        ctxT_cls_hi, ctxT_cls_lo = _hi_lo(xclsT)
        fT = np.concatenate([x_cls[:, :3].T,
                             np.ones((1, TILE))]).astype(np.float32)
        clsfT_hi, clsfT_lo = _hi_lo(fT)
        amask_cls = c_act.astype(np.float32)[:, None]
        cmask_cls = ((cid == CONTEXT_ID) & ~c_pad).astype(np.float32)[:, None]
        nonpad_cls = (~c_pad).astype(np.float32)[:, None]

        per_core.append(dict(
            slots=slots, nt=len(slots),
            a_act=a_act, a_card=a_card, a_plain=a_plain, in_cls=in_cls,
            masksT=masksT, ctxT_hi=ctxT_hi, ctxT_lo=ctxT_lo,
            masksT_cls=masksT_cls, ctxT_cls_hi=ctxT_cls_hi,
            ctxT_cls_lo=ctxT_cls_lo, clsfT_hi=clsfT_hi, clsfT_lo=clsfT_lo,
            amask_cls=amask_cls, cmask_cls=cmask_cls, nonpad_cls=nonpad_cls,
        ))
    return per_core, ntiles


def _build_tables(base_emb, street_emb, rank_emb, suit_emb, actor_emb,
                  atype_emb):
    t_card = np.concatenate([base_emb[:77], street_emb, rank_emb, suit_emb])
    t_act = np.concatenate([base_emb[:77], atype_emb, street_emb, actor_emb])
    t_plain = np.concatenate([base_emb[:77], street_emb])
    pad = lambda t: np.concatenate(
        [t, np.zeros((KMAX - t.shape[0], D), t.dtype)])
    tables = np.concatenate(
        [pad(t_card), pad(t_act), pad(t_plain)], axis=1).astype(np.float32)
    t_cls = np.concatenate([base_emb[:77], atype_emb, street_emb, rank_emb,
                            suit_emb, actor_emb]).astype(np.float32)
    return _hi_lo(tables), _hi_lo(t_cls)


def _iotas_inds():
    io_card = np.concatenate([np.arange(77), np.arange(4), np.arange(13),
                              np.arange(4)]).astype(np.float32)
    io_act = np.concatenate([np.arange(77), np.arange(60, 76), np.arange(4),
                             np.arange(2)]).astype(np.float32)
    io_plain = np.concatenate([np.arange(77), np.arange(4)]).astype(np.float32)
    io_cls = np.concatenate([np.arange(77), np.arange(60, 76), np.arange(4),
                             np.arange(13), np.arange(4),
                             np.arange(2)]).astype(np.float32)
    iota3 = np.full((KMAX, 3), -12345.0, np.float32)
    iota3[:K_CARD, 0] = io_card
    iota3[:K_ACT, 1] = io_act
    iota3[:K_PLAIN, 2] = io_plain
    iota_cls = io_cls[:, None]

    ind_card = np.zeros((4, K_CARD), NPBF)
    ind_card[0, :77] = 1
    ind_card[1, 77:81] = 1
    ind_card[2, 81:94] = 1
    ind_card[3, 94:98] = 1
    ind_act = np.zeros((3, K_ACT), NPBF)
    ind_act[1, :93] = 1        # ids: base + atype
    ind_act[2, 93:97] = 1      # street
    ind_act[0, 97:] = 1        # actor
    ind_plain = np.zeros((2, K_PLAIN), NPBF)
    ind_plain[0, :77] = 1
    ind_plain[1, 77:] = 1
    ind_cls = np.zeros((5, K_CLS), NPBF)
    ind_cls[1, :93] = 1
    ind_cls[2, 93:97] = 1
    ind_cls[3, 97:110] = 1
    ind_cls[4, 110:114] = 1
    ind_cls[0, 114:116] = 1
    return iota3, iota_cls, ind_card, ind_act, ind_plain, ind_cls


def _mlp_rhs(W, b):
    """[K+1, 512] bf16: cols 0..255 = hi([W; b]), cols 256.. = lo."""
    Wb = np.concatenate([W, b[None, :]]).astype(np.float32)
    hi, lo = _hi_lo(Wb)
    return np.ascontiguousarray(np.concatenate([hi, lo], axis=1))


def _build_bass(ntiles, nt_total, na, nx):
    nc = bacc.Bacc("TRN2", target_bir_lowering=False)

    def din(name, shape, dt=BF16):
        return nc.dram_tensor(name, shape, dt, kind="ExternalInput")

    d_a_act = din("a_act", [3, nt_total])
    d_a_card = din("a_card", [4, nt_total])
    d_a_plain = din("a_plain", [2, nt_total])
    d_in_cls = din("in_cls", [5, TILE])
    d_tab_hi = din("tab_hi", [KMAX, 3 * D])
    d_tab_lo = din("tab_lo", [KMAX, 3 * D])
    d_ctab_hi = din("ctab_hi", [K_CLS, D])
    d_ctab_lo = din("ctab_lo", [K_CLS, D])
    d_iota3 = din("iota3", [KMAX, 3], F32)
    d_iota_cls = din("iota_cls", [K_CLS, 1], F32)
    d_ind_card = din("ind_card", [4, K_CARD])
    d_ind_act = din("ind_act", [3, K_ACT])
    d_ind_plain = din("ind_plain", [2, K_PLAIN])
    d_ind_cls = din("ind_cls", [5, K_CLS])
    d_masksT = din("masksT", [17, na])
    d_ctxT_hi = din("ctxT_hi", [17, nx])
    d_ctxT_lo = din("ctxT_lo", [17, nx])
    d_legal_rhs = din("legal_rhs", [17, 2 * D])
    d_ctx_rhs = din("ctx_rhs", [17, 2 * D])
    d_cls_rhs = din("cls_rhs", [4, 2 * D])
    d_masksT_cls = din("masksT_cls", [17, TILE])
    d_ctxT_cls_hi = din("ctxT_cls_hi", [17, TILE])
    d_ctxT_cls_lo = din("ctxT_cls_lo", [17, TILE])
    d_clsfT_hi = din("clsfT_hi", [4, TILE])
    d_clsfT_lo = din("clsfT_lo", [4, TILE])
    d_amask_cls = din("amask_cls", [TILE, 1], F32)
    d_cmask_cls = din("cmask_cls", [TILE, 1], F32)
    d_nonpad_cls = din("nonpad_cls", [TILE, 1], F32)

    d_out = nc.dram_tensor("out", [nt_total, D], F32, kind="ExternalOutput")

    with tile.TileContext(nc) as tc:
        with tc.tile_pool(name="const", bufs=1) as const_p, \
             tc.tile_pool(name="work", bufs=3) as work_p, \
             tc.tile_pool(name="outp", bufs=3) as out_p, \
             tc.tile_pool(name="small", bufs=4) as small_p, \
             tc.tile_pool(name="p_inb", bufs=2, space="PSUM") as pi_p, \
             tc.tile_pool(name="p_out", bufs=2, space="PSUM") as po_p, \
             tc.tile_pool(name="p_h", bufs=2, space="PSUM") as ph_p:

            def load(d, shape, dt=BF16):
                t = const_p.tile(shape, dt, tag=d.name)
                nc.gpsimd.dma_start(out=t, in_=d.ap())
                return t

            t_in_cls = load(d_in_cls, [5, TILE])
            t_tab_hi = load(d_tab_hi, [KMAX, 3 * D])
            t_tab_lo = load(d_tab_lo, [KMAX, 3 * D])
            t_ctab_hi = load(d_ctab_hi, [K_CLS, D])
            t_ctab_lo = load(d_ctab_lo, [K_CLS, D])
            t_iota3 = load(d_iota3, [KMAX, 3], F32)
            t_iota_cls = load(d_iota_cls, [K_CLS, 1], F32)
            t_ind_card = load(d_ind_card, [4, K_CARD])
            t_ind_act = load(d_ind_act, [3, K_ACT])
            t_ind_plain = load(d_ind_plain, [2, K_PLAIN])
            t_ind_cls = load(d_ind_cls, [5, K_CLS])
            t_masksT = load(d_masksT, [17, na])
            t_ctxT_hi = load(d_ctxT_hi, [17, nx])
            t_ctxT_lo = load(d_ctxT_lo, [17, nx])
            t_legal_rhs = load(d_legal_rhs, [17, 2 * D])
            t_ctx_rhs = load(d_ctx_rhs, [17, 2 * D])
            t_cls_rhs = load(d_cls_rhs, [4, 2 * D])
            t_masksT_cls = load(d_masksT_cls, [17, TILE])
            t_ctxT_cls_hi = load(d_ctxT_cls_hi, [17, TILE])
            t_ctxT_cls_lo = load(d_ctxT_cls_lo, [17, TILE])
            t_clsfT_hi = load(d_clsfT_hi, [4, TILE])
            t_clsfT_lo = load(d_clsfT_lo, [4, TILE])
            t_amask_cls = load(d_amask_cls, [TILE, 1], F32)
            t_cmask_cls = load(d_cmask_cls, [TILE, 1], F32)
            t_nonpad_cls = load(d_nonpad_cls, [TILE, 1], F32)

            eps_t = const_p.tile([TILE, 1], F32, tag="eps")
            nc.vector.memset(eps_t, 1e-5)

            def mlp_ln(p_h, n_rows=TILE):
                """bn_stats LayerNorm; returns (rstd, nb) for fused Relu."""
                stats = small_p.tile([TILE, 6], F32, tag="stats")
                nc.vector.bn_stats(out=stats[:n_rows], in_=p_h[:n_rows, :D])
                mv = small_p.tile([TILE, 2], F32, tag="mv")
                nc.vector.bn_aggr(out=mv[:n_rows], in_=stats[:n_rows])
                std = small_p.tile([TILE, 1], F32, tag="std")
                nc.scalar.activation(out=std[:n_rows], in_=mv[:n_rows, 1:2],
                                     func=AF.Sqrt, bias=eps_t[:n_rows])
                rstd = small_p.tile([TILE, 1], F32, tag="rstd")
                nc.vector.reciprocal(out=rstd[:n_rows], in_=std[:n_rows])
                nb = small_p.tile([TILE, 1], F32, tag="nb")
                nc.vector.tensor_scalar(out=nb[:n_rows],
                                        in0=mv[:n_rows, 0:1],
                                        scalar1=rstd[:n_rows], scalar2=-1.0,
                                        op0=ALU.mult, op1=ALU.mult)
                return rstd, nb

            def mlp3(lhsT_hi, lhsT_lo, rhs2, exact_lhs):
                """h = x @ (W_hi + W_lo) [+ x_lo @ W_hi]; returns psum."""
                p_h = ph_p.tile([TILE, D], F32, tag="ph")
                m1 = nc.tensor.matmul(p_h, lhsT=lhsT_hi, rhs=rhs2[:, :D],
                                      start=True, stop=False)
                m2 = nc.tensor.matmul(p_h, lhsT=lhsT_hi, rhs=rhs2[:, D:],
                                      start=False, stop=exact_lhs)
                add_dep_helper(m2.ins, m1.ins, sync=False, reason="accum order")
                if not exact_lhs:
                    m3 = nc.tensor.matmul(p_h, lhsT=lhsT_lo, rhs=rhs2[:, :D],
                                          start=False, stop=True)
                    add_dep_helper(m3.ins, m2.ins, sync=False,
                                   reason="accum order")
                return p_h

            # ---- CLS auxiliary MLP ----
            p_hc = mlp3(t_clsfT_hi, t_clsfT_lo, t_cls_rhs, False)
            rstd, nb = mlp_ln(p_hc)
            cls_vec = const_p.tile([TILE, D], F32, tag="cls_vec")
            nc.scalar.activation(out=cls_vec, in_=p_hc[:, :D], func=AF.Relu,
                                 bias=nb, scale=rstd)

            # ---- CLS tile (tile 0) ----
            p_inb = pi_p.tile([K_CLS, GRP * TILE], F32, tag="inb")
            nc.tensor.matmul(p_inb[:K_CLS, :TILE], lhsT=t_ind_cls,
                             rhs=t_in_cls, start=True, stop=True)
            oh = work_p.tile([K_CLS, GRP * TILE], BF16, tag="oh")
            nc.vector.tensor_scalar(out=oh[:K_CLS, :TILE],
                                    in0=p_inb[:K_CLS, :TILE],
                                    scalar1=t_iota_cls, scalar2=None,
                                    op0=ALU.is_equal)
            p_out = po_p.tile([TILE, GRP * D], F32, tag="pout")
            mcls1 = nc.tensor.matmul(p_out[:, :D], lhsT=oh[:K_CLS, :TILE],
                                     rhs=t_ctab_hi, start=True, stop=False)
            mcls2 = nc.tensor.matmul(p_out[:, :D], lhsT=oh[:K_CLS, :TILE],
                                     rhs=t_ctab_lo, start=False, stop=True)
            add_dep_helper(mcls2.ins, mcls1.ins, sync=False,
                           reason="accum order")
            # action mlp (masked)
            p_h = mlp3(t_masksT_cls, None, t_legal_rhs, True)
            rstd, nb = mlp_ln(p_h)
            relu = work_p.tile([TILE, D], F32, tag="relu")
            nc.scalar.activation(out=relu, in_=p_h[:, :D], func=AF.Relu,
                                 bias=nb, scale=rstd)
            nc.vector.tensor_scalar(out=relu, in0=relu, scalar1=t_amask_cls,
                                    scalar2=None, op0=ALU.mult)
            acc = out_p.tile([TILE, GRP * D], F32, tag="out")
            nc.vector.tensor_add(acc[:, :D], p_out[:, :D], relu)
            # ctx mlp (masked)
            p_h2 = mlp3(t_ctxT_cls_hi, t_ctxT_cls_lo, t_ctx_rhs, False)
            rstd2, nb2 = mlp_ln(p_h2)
            relu2 = work_p.tile([TILE, D], F32, tag="relu")
            nc.scalar.activation(out=relu2, in_=p_h2[:, :D], func=AF.Relu,
                                 bias=nb2, scale=rstd2)
            nc.vector.tensor_scalar(out=relu2, in0=relu2, scalar1=t_cmask_cls,
                                    scalar2=None, op0=ALU.mult)
            nc.vector.tensor_add(acc[:, :D], acc[:, :D], relu2)
            nc.vector.tensor_add(acc[:, :D], acc[:, :D], cls_vec)
            nc.vector.tensor_scalar(out=acc[:, :D], in0=acc[:, :D],
                                    scalar1=t_nonpad_cls, scalar2=None,
                                    op0=ALU.mult)
            nc.sync.dma_start(out=d_out.ap()[0:TILE, :], in_=acc[:, :D])

            # ---- main segments ----
            segs = []
            off = 1
            segs.append(("plain", ntiles["plain"], off, d_a_plain, 2,
                         t_ind_plain, t_iota3[:K_PLAIN, 2:3], K_PLAIN,
                         t_tab_hi[:K_PLAIN, 2 * D:], t_tab_lo[:K_PLAIN, 2 * D:],
                         None, None, None))
            off += ntiles["plain"]
            segs.append(("card", ntiles["card"], off, d_a_card, 4,
                         t_ind_card, t_iota3[:K_CARD, 0:1], K_CARD,
                         t_tab_hi[:K_CARD, :D], t_tab_lo[:K_CARD, :D],
                         None, None, None))
            off += ntiles["card"]
            segs.append(("act", ntiles["act"], off, d_a_act, 3,
                         t_ind_act, t_iota3[:K_ACT, 1:2], K_ACT,
                         t_tab_hi[:K_ACT, D:2 * D], t_tab_lo[:K_ACT, D:2 * D],
                         t_masksT, None, t_legal_rhs))
            off += ntiles["act"]
            segs.append(("ctx", ntiles["ctx"], off, d_a_plain, 2,
                         t_ind_plain, t_iota3[:K_PLAIN, 2:3], K_PLAIN,
                         t_tab_hi[:K_PLAIN, 2 * D:], t_tab_lo[:K_PLAIN, 2 * D:],
                         t_ctxT_hi, t_ctxT_lo, t_ctx_rhs))

            for (name, n_t, t_off, d_a, nin, ind_t, iota_ap, K, tab_hi,
                 tab_lo, mlpT_hi, mlpT_lo, mlp_rhs) in segs:
                for g0 in range(0, n_t, GRP):
                    gn = min(GRP, n_t - g0)
                    w = gn * TILE
                    col0 = (t_off + g0) * TILE
                    g_in = work_p.tile([5, GRP * TILE], BF16, tag="gin")
                    nc.gpsimd.dma_start(out=g_in[:nin, :w],
                                        in_=d_a.ap()[0:nin, col0:col0 + w])
                    p_inb = pi_p.tile([K_CLS, GRP * TILE], F32, tag="inb")
                    nc.tensor.matmul(p_inb[:K, :w], lhsT=ind_t,
                                     rhs=g_in[:nin, :w], start=True, stop=True)
                    oh = work_p.tile([K_CLS, GRP * TILE], BF16, tag="oh")
                    nc.vector.tensor_scalar(out=oh[:K, :w], in0=p_inb[:K, :w],
                                            scalar1=iota_ap, scalar2=None,
                                            op0=ALU.is_equal)
                    p_out = po_p.tile([TILE, GRP * D], F32, tag="pout")
                    o_sb = out_p.tile([TILE, GRP * D], F32, tag="out")
                    # all hi passes first, then all lo passes: the >=3-matmul
                    # gap hides the PSUM read-modify-write stall of
                    # accumulating (start=False) matmuls.
                    prev_mm = None
                    for phase, tab in ((0, tab_hi), (1, tab_lo)):
                        for i in range(gn):
                            osl = slice(i * D, (i + 1) * D)
                            ohsl = oh[:K, i * TILE:(i + 1) * TILE]
                            # start=True only on the first matmul touching a
                            # PSUM bank (2 tiles per 2KB bank): it clears the
                            # whole bank's has_written bits.
                            st = phase == 0 and i % 2 == 0
                            mm = nc.tensor.matmul(p_out[:, osl], lhsT=ohsl,
                                                  rhs=tab, start=st,
                                                  stop=(phase == 1),
                                                  skip_group_check=True)
                            if prev_mm is not None:
                                add_dep_helper(mm.ins, prev_mm.ins,
                                               sync=False,
                                               reason="accum order")
                            prev_mm = mm
                    if mlp_rhs is None:
                        nc.scalar.activation(out=o_sb[:, :w * 2],
                                             in_=p_out[:, :w * 2],
                                             func=AF.Copy)
                    else:
                        for i0 in range(0, gn, 2):
                            pn = min(2, gn - i0)
                            phs, prev = [], None
                            for ph_phase in range(3 if mlpT_lo is not None
                                                  else 2):
                                for j in range(pn):
                                    t = g0 + i0 + j
                                    tsl = slice(t * TILE, (t + 1) * TILE)
                                    if ph_phase == 0:
                                        p_h = ph_p.tile([TILE, D], F32,
                                                        tag="ph")
                                        phs.append(p_h)
                                        mm = nc.tensor.matmul(
                                            p_h, lhsT=mlpT_hi[:, tsl],
                                            rhs=mlp_rhs[:, :D],
                                            start=True, stop=False)
                                    elif ph_phase == 1:
                                        mm = nc.tensor.matmul(
                                            phs[j], lhsT=mlpT_hi[:, tsl],
                                            rhs=mlp_rhs[:, D:],
                                            start=False,
                                            stop=mlpT_lo is None)
                                    else:
                                        mm = nc.tensor.matmul(
                                            phs[j], lhsT=mlpT_lo[:, tsl],
                                            rhs=mlp_rhs[:, :D],
                                            start=False, stop=True)
                                    if prev is not None:
                                        add_dep_helper(mm.ins, prev.ins,
                                                       sync=False,
                                                       reason="accum order")
                                    prev = mm
                            for j in range(pn):
                                i = i0 + j
                                osl = slice(i * D, (i + 1) * D)
                                rstd, nb = mlp_ln(phs[j])
                                relu = work_p.tile([TILE, D], F32, tag="relu")
                                nc.scalar.activation(out=relu,
                                                     in_=phs[j][:, :D],
                                                     func=AF.Relu, bias=nb,
                                                     scale=rstd)
                                nc.vector.tensor_add(o_sb[:, osl],
                                                     p_out[:, osl], relu)
                    row0 = (t_off + g0) * TILE
                    src = o_sb[:, :w * 2].rearrange("p (g d) -> p g d", g=gn)
                    dst = d_out.ap()[row0:row0 + gn * TILE, :].rearrange(
                        "(g p) d -> p g d", p=TILE)
                    nc.sync.dma_start(out=dst, in_=src)

    if not nc.is_finalized():
        nc.finalize()
    return nc


def kernel(token_ids, token_streets, card_ranks, card_suits, action_actors,
           action_legal_masks, context_features,
           base_emb, street_emb, rank_emb, suit_emb, actor_emb, atype_emb,
           legal_W, legal_b, legal_g, legal_be,
           cls_W, cls_b, cls_g, cls_be,
           ctx_W, ctx_b, ctx_g, ctx_be, _trace=False):
    per_core, ntiles = _build_host_data(
        np.asarray(token_ids), np.asarray(token_streets),
        np.asarray(card_ranks), np.asarray(card_suits),
        np.asarray(action_actors), np.asarray(action_legal_masks),
        np.asarray(context_features))
    nt_total = per_core[0]["nt"]
    na = ntiles["act"] * TILE
    nx = ntiles["ctx"] * TILE

    for g, be in ((legal_g, legal_be), (cls_g, cls_be), (ctx_g, ctx_be)):
        assert np.allclose(np.asarray(g), 1.0) and np.allclose(
            np.asarray(be), 0.0), "non-trivial LN affine not supported"

    (tab_hi, tab_lo), (ctab_hi, ctab_lo) = _build_tables(
        np.asarray(base_emb), np.asarray(street_emb), np.asarray(rank_emb),
        np.asarray(suit_emb), np.asarray(actor_emb), np.asarray(atype_emb))
    iota3, iota_cls, ind_card, ind_act, ind_plain, ind_cls = _iotas_inds()
    legal_rhs = _mlp_rhs(np.asarray(legal_W), np.asarray(legal_b))
    ctx_rhs = _mlp_rhs(np.asarray(ctx_W), np.asarray(ctx_b))
    cls_rhs = _mlp_rhs(np.asarray(cls_W), np.asarray(cls_b))

    nc = _build_bass(ntiles, nt_total, na, nx)

    shared = dict(tab_hi=tab_hi, tab_lo=tab_lo, ctab_hi=ctab_hi,
                  ctab_lo=ctab_lo, iota3=iota3, iota_cls=iota_cls,
                  ind_card=ind_card, ind_act=ind_act, ind_plain=ind_plain,
                  ind_cls=ind_cls, legal_rhs=legal_rhs, ctx_rhs=ctx_rhs,
                  cls_rhs=cls_rhs)
    in_maps = []
    for c in range(NCORES):
        pc = per_core[c]
        im = dict(shared)
        im.update(a_act=pc["a_act"], a_card=pc["a_card"],
                  a_plain=pc["a_plain"], in_cls=pc["in_cls"],
                  masksT=pc["masksT"], ctxT_hi=pc["ctxT_hi"],
                  ctxT_lo=pc["ctxT_lo"], masksT_cls=pc["masksT_cls"],
                  ctxT_cls_hi=pc["ctxT_cls_hi"],
                  ctxT_cls_lo=pc["ctxT_cls_lo"], clsfT_hi=pc["clsfT_hi"],
                  clsfT_lo=pc["clsfT_lo"], amask_cls=pc["amask_cls"],
                  cmask_cls=pc["cmask_cls"], nonpad_cls=pc["nonpad_cls"])
        in_maps.append({k: np.ascontiguousarray(v) for k, v in im.items()})

    res = run_bass_kernel_spmd(nc, in_maps, core_ids=list(range(NCORES)),
                               trace=_trace)
    if _trace:
        print(f"HW exec time: {res.exec_time_ns} ns")
        print(f"mean exec time: {res.mean_exec_time_ns} ns")
        if res.instructions_and_trace:
            print("trace:", res.instructions_and_trace[1])

    full = np.zeros((B * S, D), np.float32)
    for c in range(NCORES):
        out_c = res.results[c]["out"]
        slots = per_core[c]["slots"]
        valid = slots >= 0
        full[slots[valid]] = out_c[valid]
    return full.reshape(B, S, D)

